# revision 25
# baseline (speedup 1.0000x reference)
"""Bass kernel builder + host prep for nn_ContextualAttention on 8 trn2 cores.

Sharding: core = 2*s + q (s = sample 0..3, q = lf-half 0..1).
Window: 30 grid-row positions pi in [0,30), true row t(pi) = pi - 3 + 24q.
Score cols: [0,1440) window, [1440,1536) far_top, [1536,1632) far_bot.
Consumed (softmax/recon) cols: window pi in [2,28) -> global [96,1344), NA=1248.

v2: no SBUF->SBUF shift DMAs. All partition shifts (diagonal fuse pass-1/2,
far corrections) are tensor-engine permutation matmuls into PSUM; hosts ships
shift matrices. slab1 holds Sn (f16, 1632 cols, E bf16 overlays cols [0,1248)
after pass-1); slab2 holds S1 for exactly the consumed sources: cols [0,1344)
= global [48,1392), [1344,1392) = far_top [1441,1488), [1392,1440) = far_bot
[1584,1631). rden moved to host (rdent input). wt/rawt layouts are dense per
partition.
"""
import numpy as np
import ml_dtypes
import contextlib
import concourse.bass as bass
from concourse import bacc, bass_isa
import concourse.tile as tile
from concourse import mybir

F16 = mybir.dt.float16
F32 = mybir.dt.float32
BF16 = mybir.dt.bfloat16
AL = mybir.AluOpType
AF = mybir.ActivationFunctionType

G = 48
J = 18
KT = 9
LB = 2304
C = 128
WINP = 30
WIN = WINP * G          # 1440
FT0 = WIN               # 1440
FB0 = WIN + 96          # 1536
NCOL = WIN + 192        # 1632
NA = 26 * G             # 1248
ESC_BIAS = 1152 * 1e-4

# shift-matrix indices in shm input
UP1, DN1, CUP1, CDN1, UP48, DN48, CUP48, CDN48, P79, PM79 = range(10)

# scores GEMM chunks over slab1 cols
SCORE_CHUNKS = ((0, 512), (512, 512), (1024, 512), (1536, 96))
# pass-1 chunks over slab2 cols (far sub-blocks only need 47 cols each;
# slab2 cols 1391 and 1439 stay unwritten/unread). Far chunks first so
# pass-2 chunk 0 (which reads the far-bot cols) unblocks earliest.
P1_CHUNKS = ((1344, 47), (1392, 47), (0, 512), (512, 512), (1024, 320))
# pass-2 chunks: (global col start, width); local d0 = g0 - 96
P2_CHUNKS = ((96, 432), (528, 432), (960, 384))
# far correction target global col ranges (47 wide each)
B_LO, B_HI = 26 * G, 26 * G + G - 1        # ftP/addC targets (chunk 2)
BP_LO, BP_HI = 3 * G + 1, 3 * G + G        # fbM/addCp targets (chunk 0)


def s2g(c):
    """slab2 col -> slab1 (global score) col."""
    if c < 1344:
        return c + 48
    if c < 1392:
        return c - 1344 + 1441
    return c - 1392 + 1584


def build(debug=False):
    nc = bacc.Bacc()
    fp_d = nc.dram_tensor("fp", [KT, 128, NCOL], F16, kind="ExternalInput")
    wt_d = nc.dram_tensor("wt", [J, 128, KT * 128], F16, kind="ExternalInput")
    rawt_d = nc.dram_tensor("rawt", [16, 128, J * 128], BF16, kind="ExternalInput")
    shm_d = nc.dram_tensor("shm", [10, 128, 128], F16, kind="ExternalInput")
    rdent_d = nc.dram_tensor("rdent", [128, J], F32, kind="ExternalInput")
    s10_d = nc.dram_tensor("s10", [128, J], F32, kind="ExternalInput")
    gcol_d = nc.dram_tensor("gcol", [1, NA], F32, kind="ExternalInput")
    gate_d = nc.dram_tensor("gate", [128, 2], F32, kind="ExternalInput")
    out_d = nc.dram_tensor("out", [128, 48, 96], F32, kind="ExternalOutput")
    if debug:
        dSn_d = nc.dram_tensor("dSn", [128, J, NCOL], F16, kind="ExternalOutput")
        dS1_d = nc.dram_tensor("dS1", [128, J, 1440], F16, kind="ExternalOutput")
        dE_d = nc.dram_tensor("dE", [128, J, NA], BF16, kind="ExternalOutput")
        dZ_d = nc.dram_tensor("dZ", [1, NA], F32, kind="ExternalOutput")

    with tile.TileContext(nc) as tc, contextlib.ExitStack() as ctx:
        consts = ctx.enter_context(tc.tile_pool(name="consts", bufs=1))
        wtp = ctx.enter_context(tc.tile_pool(name="wtp", bufs=2))
        big = ctx.enter_context(tc.tile_pool(name="big", bufs=1))
        lpool = ctx.enter_context(tc.tile_pool(name="lpool", bufs=1))
        work = ctx.enter_context(tc.tile_pool(name="work", bufs=1))
        rawp = ctx.enter_context(tc.tile_pool(name="rawp", bufs=2))
        gsp = ctx.enter_context(tc.tile_pool(name="gsp", bufs=2))

        # fpt + first wt loads first: they gate the first matmul
        fpt = big.tile([128, KT, NCOL], F16, tag="U1")
        for o in range(KT):
            nc.sync.dma_start(out=fpt[:, o, :], in_=fp_d[o, :, :])

        # ---------------- consts / small inputs ----------------
        s10t = consts.tile([128, J], F32, tag="s10t")
        nc.sync.dma_start(out=s10t, in_=s10_d[:, :])
        gcolt = consts.tile([1, NA], F32, tag="gcolt")
        nc.sync.dma_start(out=gcolt, in_=gcol_d[:, :])
        gatet = consts.tile([128, 2], F32, tag="gatet")
        nc.sync.dma_start(out=gatet, in_=gate_d[:, :])
        rdent = consts.tile([128, J], F32, tag="rdent")
        nc.sync.dma_start(out=rdent, in_=rdent_d[:, :])
        shmt = consts.tile([128, 10, 128], F16, tag="shmt")
        for i in range(10):
            nc.sync.dma_start(out=shmt[:, i, :], in_=shm_d[i, :, :])
        ones16 = consts.tile([128, 1], BF16, tag="ones16")
        nc.vector.memset(ones16, 1.0)

        def shmat(i):
            return shmt[:, i, :]

        # ---------------- scores GEMM -> slab1 (Sn, f16) ----------------
        slab1 = big.tile([128, J, NCOL], F16, tag="slab1")
        with tc.tile_pool(name="psc", bufs=4, space="PSUM") as psc:
            for j in range(J):
                wtj = wtp.tile([128, KT * 128], F16, tag="wtj")
                nc.sync.dma_start(out=wtj, in_=wt_d[j, :, :])
                for c0, w in SCORE_CHUNKS:
                    ps = psc.tile([128, 512], F32, tag="sps")
                    for o in range(KT):
                        nc.tensor.matmul(ps[:, 0:w], wtj[:, o * 128:(o + 1) * 128],
                                         fpt[:, o, c0:c0 + w],
                                         start=(o == 0), stop=(o == KT - 1))
                    nc.scalar.activation(slab1[:, j, c0:c0 + w], ps[:, 0:w],
                                         AF.Copy, scale=rdent[:, j:j + 1])
        if debug:
            nc.sync.dma_start(out=dSn_d[:, :, :], in_=slab1)

        # ---------------- pass-1: S1 = Sn + diag(+1) + diag(-1) -> slab2 ----
        # slab2 shares the U1 slot with fpt (dead after scores GEMM) and
        # out_acc (recon starts after pass-2 ends)
        slab2 = big.tile([128, J, 1440], F16, tag="U1")
        with tc.tile_pool(name="psp1", bufs=4, space="PSUM") as psp1:
            for c0, w in P1_CHUNKS:
                g0 = s2g(c0)
                for j in range(J):
                    ps = psp1.tile([128, 512], F32, tag="p1ps")
                    nc.tensor.matmul(ps[:, 0:w], shmat(UP1),
                                     slab1[:, j, g0 + 1:g0 + 1 + w],
                                     start=True, stop=False)
                    if j < J - 1:
                        nc.tensor.matmul(ps[:, 0:w], shmat(CUP1),
                                         slab1[:, j + 1, g0 + 1:g0 + 1 + w],
                                         start=False, stop=False)
                    if j > 0:
                        nc.tensor.matmul(ps[:, 0:w], shmat(CDN1),
                                         slab1[:, j - 1, g0 - 1:g0 - 1 + w],
                                         start=False, stop=False)
                    nc.tensor.matmul(ps[:, 0:w], shmat(DN1),
                                     slab1[:, j, g0 - 1:g0 - 1 + w],
                                     start=False, stop=True)
                    nc.vector.tensor_tensor(out=slab2[:, j, c0:c0 + w],
                                            in0=ps[:, 0:w],
                                            in1=slab1[:, j, g0:g0 + w], op=AL.add)
                # gates right after the producing chunk: zero pi=2 block
                # (q=0) / pi=27 block (q=1); slab2 cols = g-48
                if c0 == 0:
                    nc.vector.tensor_scalar_mul(slab2[:, :, 48:96],
                                                slab2[:, :, 48:96], gatet[:, 0:1])
                if c0 == 1024:
                    nc.vector.tensor_scalar_mul(slab2[:, :, 1248:1296],
                                                slab2[:, :, 1248:1296],
                                                gatet[:, 1:2])
        if debug:
            nc.sync.dma_start(out=dS1_d[:, :, :], in_=slab2)

        # E overlays slab1 cols [0, NA) as bf16 (Sn dead after pass-1)
        Ebig = slab1[:, :, 0:NA].bitcast(BF16)
        Zrow = consts.tile([1, NA], F32, tag="Zrow")

        # ---------------- pass-2 + softmax, interleaved with recon ----------
        rzb = consts.tile([128, NA], F32, tag="rzb")
        rzrow = consts.tile([1, NA], F32, tag="rzrow")
        with tc.tile_pool(name="psp2", bufs=4, space="PSUM") as psp2, \
             tc.tile_pool(name="psz", bufs=1, space="PSUM") as psz, \
             tc.tile_pool(name="psg", bufs=3, space="PSUM") as psg:

            def emit_pass2_chunk(ci):
                g0, w = P2_CHUNKS[ci]
                d0 = g0 - 96
                c0 = g0 - 48                       # slab2 col of g0
                Lt = lpool.tile([128, J, 432], F32, tag="Lt")
                for j in range(J):
                    ps = psp2.tile([128, 432], F32, tag="p2ps")
                    nc.tensor.matmul(ps[:, 0:w], shmat(UP48),
                                     slab2[:, j, c0 + 48:c0 + 48 + w],
                                     start=True, stop=False)
                    if j < J - 1:
                        nc.tensor.matmul(ps[:, 0:w], shmat(CUP48),
                                         slab2[:, j + 1, c0 + 48:c0 + 48 + w],
                                         start=False, stop=False)
                    if j > 0:
                        nc.tensor.matmul(ps[:, 0:w], shmat(CDN48),
                                         slab2[:, j - 1, c0 - 48:c0 - 48 + w],
                                         start=False, stop=False)
                    # row-wrap terms (by=47 up-wrap at j=17, by=0 dn-wrap at j=0)
                    if j == J - 1:
                        nc.tensor.matmul(ps[:, 0:w], shmat(P79),
                                         slab2[:, 0, c0 + 48:c0 + 48 + w],
                                         start=False, stop=False)
                    if j == 0:
                        nc.tensor.matmul(ps[:, 0:w], shmat(PM79),
                                         slab2[:, J - 1, c0 - 48:c0 - 48 + w],
                                         start=False, stop=False)
                    if ci == 2:
                        # B targets [B_LO, B_HI): psum cols, ft sources
                        a, b = B_LO - g0, B_HI - g0
                        nw = b - a
                        nc.tensor.matmul(ps[:, a:b], shmat(UP48),
                                         slab2[:, j, 1344:1344 + nw],
                                         start=False, stop=False)
                        if j < J - 1:
                            nc.tensor.matmul(ps[:, a:b], shmat(CUP48),
                                             slab2[:, j + 1, 1344:1344 + nw],
                                             start=False, stop=False)
                        if j == J - 1:
                            nc.tensor.matmul(ps[:, a:b], shmat(P79),
                                             slab2[:, 0, 1344:1344 + nw],
                                             start=False, stop=False)
                    if ci == 0:
                        # B' targets [BP_LO, BP_HI): fb sources
                        a, b = BP_LO - g0, BP_HI - g0
                        nw = b - a
                        nc.tensor.matmul(ps[:, a:b], shmat(DN48),
                                         slab2[:, j, 1392:1392 + nw],
                                         start=False, stop=False)
                        if j > 0:
                            nc.tensor.matmul(ps[:, a:b], shmat(CDN48),
                                             slab2[:, j - 1, 1392:1392 + nw],
                                             start=False, stop=False)
                        if j == 0:
                            nc.tensor.matmul(ps[:, a:b], shmat(PM79),
                                             slab2[:, J - 1, 1392:1392 + nw],
                                             start=False, stop=False)
                    nc.tensor.matmul(ps[:, 0:w], shmat(DN48),
                                     slab2[:, j, c0 - 48:c0 - 48 + w],
                                     start=False, stop=True)
                    # S2 = psum + S1, then L = S2 * s10 (scalar engine)
                    nc.vector.tensor_tensor(out=Lt[:, j, 0:w], in0=ps[:, 0:w],
                                            in1=slab2[:, j, c0:c0 + w], op=AL.add)
                    nc.scalar.activation(Lt[:, j, 0:w], Lt[:, j, 0:w],
                                         AF.Copy, scale=s10t[:, j:j + 1])
                # max over lb: tree over j on gpsimd (frees vector for the
                # next chunk's psum-drain adds), then across partitions
                t9 = work.tile([128, 9, 432], F32, tag="shA")
                nc.vector.tensor_tensor(out=t9[:, :, 0:w], in0=Lt[:, 0:9, 0:w],
                                        in1=Lt[:, 9:18, 0:w], op=AL.max)
                t4 = work.tile([128, 4, 432], F32, tag="shB")
                nc.vector.tensor_tensor(out=t4[:, :, 0:w], in0=t9[:, 0:4, 0:w],
                                        in1=t9[:, 4:8, 0:w], op=AL.max)
                t2 = work.tile([128, 2, 432], F32, tag="t2")
                nc.vector.tensor_tensor(out=t2[:, :, 0:w], in0=t4[:, 0:2, 0:w],
                                        in1=t4[:, 2:4, 0:w], op=AL.max)
                mx = work.tile([128, 432], F32, tag="mx")
                nc.vector.tensor_tensor(out=mx[:, 0:w], in0=t2[:, 0, 0:w],
                                        in1=t2[:, 1, 0:w], op=AL.max)
                nc.vector.tensor_tensor(out=mx[:, 0:w], in0=mx[:, 0:w],
                                        in1=t9[:, 8, 0:w], op=AL.max)
                mxb = work.tile([128, 432], F32, tag="mxb")
                nc.gpsimd.partition_all_reduce(mxb[:, 0:w], mx[:, 0:w],
                                               channels=128,
                                               reduce_op=bass_isa.ReduceOp.max)
                mview = bass.AP(tensor=mxb.tensor, offset=mxb.offset,
                                ap=[mxb.ap[0], [0, J], [1, w]])
                nc.vector.tensor_tensor(out=Lt[:, :, 0:w], in0=Lt[:, :, 0:w],
                                        in1=mview, op=AL.subtract)
                # E = exp(u) -> bf16 overlay
                nc.scalar.activation(Ebig[:, :, d0:d0 + w], Lt[:, :, 0:w], AF.Exp)
                # Z = ones^T E (before mask-zeroing)
                zp = psz.tile([1, 432], F32, tag="zp")
                for j in range(J):
                    nc.tensor.matmul(zp[:, 0:w], ones16, Ebig[:, j, d0:d0 + w],
                                     start=(j == 0), stop=(j == J - 1))
                nc.scalar.activation(Zrow[:, d0:d0 + w], zp[:, 0:w], AF.Copy)
                # per-chunk recipZ = 0.25 * gcol / Z, broadcast to all partitions
                nc.vector.reciprocal(rzrow[:, d0:d0 + w], Zrow[:, d0:d0 + w])
                nc.vector.tensor_tensor(out=rzrow[:, d0:d0 + w],
                                        in0=rzrow[:, d0:d0 + w],
                                        in1=gcolt[:, d0:d0 + w], op=AL.mult)
                nc.gpsimd.partition_broadcast(rzb[:, d0:d0 + w], rzrow[:, d0:d0 + w])

            ky_pis = {0: (4, 28), 1: (3, 27), 2: (3, 27), 3: (2, 26)}
            kx_us = {0: (1, 48), 1: (0, 48), 2: (0, 48), 3: (0, 47)}
            rchunk_off = [0, 432, 864]
            RCH = (432, 432, 384)

            def emit_recon_tap(tap):
                ky, kx = tap // 4, tap % 4
                plo, phi = ky_pis[ky]
                ulo, uhi = kx_us[kx]
                rawtile = rawp.tile([128, J * 128], BF16, tag="rawtile")
                nc.sync.dma_start(out=rawtile, in_=rawt_d[tap, :, :])
                for ri, rw in enumerate(RCH):
                    r0 = rchunk_off[ri]
                    cplo = 2 + r0 // G
                    cphi = 2 + (r0 + rw) // G
                    a = max(plo, cplo); bnd = min(phi, cphi)
                    if a >= bnd:
                        continue
                    # trim matmul/scale to used pi blocks
                    A = (a - 2) * G - r0
                    B2 = (bnd - 2) * G - r0
                    gp = psg.tile([128, 432], F32, tag="gp")
                    for j in range(J):
                        nc.tensor.matmul(gp[:, A:B2], rawtile[:, j * 128:(j + 1) * 128],
                                         Ebig[:, j, r0 + A:r0 + B2],
                                         start=(j == 0), stop=(j == J - 1))
                    gs = gsp.tile([128, 432], BF16, tag="gs")
                    nc.vector.tensor_tensor(out=gs[:, A:B2], in0=gp[:, A:B2],
                                            in1=rzb[:, r0 + A:r0 + B2], op=AL.mult)
                    npi = bnd - a
                    nu = uhi - ulo
                    goff = (a - 2) * G + ulo - r0
                    gview = bass.AP(tensor=gs.tensor, offset=gs.offset + goff,
                                    ap=[gs.ap[0], [G, npi], [1, nu]])
                    yl0 = 2 * (a - 3) + ky - 1
                    xl0 = 2 * ulo + kx - 1
                    oview = bass.AP(tensor=out_acc.tensor,
                                    offset=out_acc.offset + yl0 * 96 + xl0,
                                    ap=[out_acc.ap[0], [192, npi], [2, nu]])
                    eng = nc.gpsimd if (tap % 2 == 0) else nc.vector
                    eng.tensor_tensor(out=oview, in0=oview, in1=gview, op=AL.add)

            emit_pass2_chunk(0)
            emit_pass2_chunk(1)
            emit_pass2_chunk(2)
            out_acc = big.tile([128, 48, 96], F32, tag="U1")
            nc.vector.memset(out_acc, 0.0)
            for _tap in range(16):
                emit_recon_tap(_tap)
        if debug:
            nc.sync.dma_start(out=dE_d[:, :, :], in_=Ebig)
            nc.sync.dma_start(out=dZ_d[:, :], in_=Zrow)
        nc.sync.dma_start(out=out_d[:, :, :], in_=out_acc)
    nc.finalize()
    return nc


# ======================= host side =======================

def make_shift_mats():
    m = np.zeros((10, 128, 128), np.float16)
    m[UP1] = np.eye(128, k=-1)     # [k,m]: k=m+1
    m[DN1] = np.eye(128, k=1)      # k=m-1
    m[CUP1, 0, 127] = 1.0
    m[CDN1, 127, 0] = 1.0
    m[UP48] = np.eye(128, k=-48)   # k=m+48 (m<=79)
    m[DN48] = np.eye(128, k=48)    # k=m-48 (m>=48)
    m[CUP48] = np.eye(128, k=80)   # k=m-80 (m in 80..127)
    m[CDN48] = np.eye(128, k=-80)  # k=m+80 (m in 0..47)
    p79 = np.eye(128, k=79)        # m=k+79
    p79[0, :] = 0.0
    p79[48:, :] = 0.0              # keep k in [1,47]
    m[P79] = p79
    pm79 = np.eye(128, k=-79)      # m=k-79
    pm79[79, :] = 0.0
    pm79[127, :] = 0.0             # keep k in [80,126]
    m[PM79] = pm79
    return m


def prep_core_inputs(f, b, mask):
    """Full inputs -> list of 8 in_map dicts (core = 2*s + q)."""
    B = f.shape[0]
    ms = np.pad(mask[0][:, ::8, ::8][0], 1)
    w = np.lib.stride_tricks.sliding_window_view(ms, (3, 3))
    mm = (w.sum((2, 3)) == 0).astype(np.float32).reshape(LB)
    s10 = np.ascontiguousarray((10.0 * mm).reshape(J, 128).T)
    mbin = np.ascontiguousarray(mm.reshape(J, 128).T)
    shm = make_shift_mats()
    in_maps = []
    for s in range(B):
        fs = f[s][:, ::2, ::2]
        bs = b[s][:, ::2, ::2]
        fsp = np.pad(fs, ((0, 0), (1, 1), (1, 1)))
        bsp = np.pad(bs, ((0, 0), (1, 1), (1, 1)))
        bhwc = np.pad(b[s], ((0, 0), (1, 1), (1, 1))).transpose(1, 2, 0)
        wt = np.empty((KT, C, LB), np.float32)
        for o in range(KT):
            dy, dx = o // 3, o % 3
            wt[o] = bsp[:, dy:dy + G, dx:dx + G].reshape(C, LB)
        # rdent: 1/sqrt(patch sum of squares + 1152*eps), [128, J]
        ssq = np.zeros((G + 2, G + 2), np.float32)
        ssq[1:G + 1, 1:G + 1] = (bs * bs).sum(0)
        sw = np.lib.stride_tricks.sliding_window_view(ssq, (3, 3)).sum((2, 3))
        rd = 1.0 / np.sqrt(sw.reshape(LB) + ESC_BIAS)
        rdent = np.ascontiguousarray(rd.reshape(J, 128).T).astype(np.float32)
        # wt layout [J, C, KT*128]: [j, c, o*128+m] = wt[o, c, j*128+m]
        wt_blocks = np.ascontiguousarray(
            wt.reshape(KT, C, J, 128).transpose(2, 1, 0, 3).reshape(J, C, KT * 128)
        ).astype(np.float16)
        iy, ix = np.divmod(np.arange(LB), G)
        rawt = np.empty((16, LB, C), np.float32)
        for ky in range(4):
            for kx in range(4):
                rawt[ky * 4 + kx] = bhwc[2 * iy + ky, 2 * ix + kx, :]
        rawt *= mm[None, :, None]      # masked rows contribute 0 to recon
        # rawt layout [16, 128, J*128]: [tap, p, j*128+c] = rawt[tap, j*128+p, c]
        rawt_blocks = np.ascontiguousarray(
            rawt.reshape(16, J, 128, C).transpose(0, 2, 1, 3).reshape(16, 128, J * C)
        ).astype(ml_dtypes.bfloat16)
        for q in (0, 1):
            ts_ = np.arange(WINP) - 3 + 24 * q
            fcols = np.zeros((KT, C, NCOL), np.float32)
            valid = (ts_ >= 0) & (ts_ < G)
            for o in range(KT):
                dy, dx = o // 3, o % 3
                block = fsp[:, (ts_ + dy).clip(0, G + 1), :][:, :, dx:dx + G]
                block = block * valid[None, :, None]
                fcols[o, :, :WIN] = block.reshape(C, WIN)
                if q == 1:
                    fcols[o, :, FT0:FT0 + 96] = fsp[:, dy:dy + 2, dx:dx + G].reshape(C, 96)
                else:
                    fcols[o, :, FB0:FB0 + 96] = fsp[:, 46 + dy:48 + dy, dx:dx + G].reshape(C, 96)
            gate = np.zeros((128, 2), np.float32)
            gate[:, 0] = 0.0 if q == 0 else 1.0
            gate[:, 1] = 1.0 if q == 0 else 0.0
            gcol = np.full((1, NA), 0.25, np.float32)   # 0.25 recon scale folded in
            if q == 0:
                gcol[0, 0:G] = 0.0
            else:
                gcol[0, NA - G:NA] = 0.0
            in_maps.append(dict(
                fp=fcols.astype(np.float16),
                wt=wt_blocks,
                rawt=rawt_blocks,
                shm=shm,
                rdent=rdent,
                s10=s10, gcol=gcol, gate=gate,
            ))
    return in_maps


def assemble(results, B=4):
    out = np.zeros((B, C, 96, 96), np.float32)
    for s in range(B):
        for q in (0, 1):
            out[s, :, 48 * q:48 * q + 48, :] = results[2 * s + q]["out"]
    return out


# ======================= self-contained runner =======================
# kernel(**inputs) entry point: full inputs in, full output out.
_NC_CACHE = {}
last_exec_time_ns = None
last_result = None


def kernel(f, b, mask):
    global last_exec_time_ns, last_result
    import os
    from concourse.bass_utils import run_bass_kernel_spmd
    f = np.ascontiguousarray(np.asarray(f, dtype=np.float32))
    b = np.ascontiguousarray(np.asarray(b, dtype=np.float32))
    mask = np.ascontiguousarray(np.asarray(mask, dtype=np.float32))
    in_maps = prep_core_inputs(f, b, mask)
    if "nc" not in _NC_CACHE:
        _NC_CACHE["nc"] = build(debug=False)
    nc = _NC_CACHE["nc"]
    trace = bool(os.environ.get("BASS_TRACE"))
    res = run_bass_kernel_spmd(nc, in_maps, core_ids=list(range(8)), trace=trace)
    last_result = res
    last_exec_time_ns = res.exec_time_ns
    return assemble([res.results[i] for i in range(8)], B=f.shape[0])
